# revision 16
# baseline (speedup 1.0000x reference)
"""Trainium2 Bass kernel for nn_DiffusionModel1d (batched 1-D diffusion solve).

Math: the reference solves A(K) u = f per batch row with K = exp(x) via the
Thomas algorithm, where A = G^T diag(K_hat) G, G the n x n lower-bidiagonal
difference matrix (1 on diag, -1 on subdiag) and
K_hat = (2*K_0, K_1, ..., K_{n-1}).  Hence

    u = h2 * G^{-1} diag(K_hat)^{-1} G^{-T} f
      = h2 * cumsum_j( w_j * exp(-x_j) ),   w = suffix_sum(f), w_0 halved.

So the whole solve is: one exp, one elementwise multiply by a shared
per-column vector, and one hardware prefix-sum scan along the grid dim.
Pure data parallel over batch: 8192 rows -> 1024 rows per core x 8 cores.
The tiny shared w vector (2047 elements, derived from the replicated f_rhs
by one suffix-sum pass) is prepared host-side and shipped replicated across
the 128 SBUF partitions, so the device pipeline has no serial prologue.

Engine budget per core (measured): DVE 2x bf16 mult 1.2us + 2 chained fp32
scans 2x2.3us per 128-row group (x8 groups), ACT exp ~1.5us, DMA 16.8 MB.
"""

import os
import sys

import numpy as np

sys.path.insert(0, "/opt/trn_rl_repo")

import ml_dtypes

import concourse.bacc as bacc
import concourse.mybir as mybir
import concourse.tile as tile
from concourse import bass_utils

B, M = 8192, 2048
N = M - 1
NCORES = 8
BC = B // NCORES          # 1024 batch rows per core
P = 128                   # SBUF partitions
GROUPS = BC // P          # 8 partition-groups per core
H2 = (1.0 / N) ** 2

_cached_nc = None
LAST_RESULTS = None


def _build_kernel():
    fp32 = mybir.dt.float32
    bf16 = mybir.dt.bfloat16
    nc = bacc.Bacc(
        "TRN2",
        target_bir_lowering=False,
        debug=False,
        enable_asserts=False,
        num_devices=NCORES,
    )
    x_d = nc.dram_tensor("x", (BC, M), fp32, kind="ExternalInput").ap()
    w_d = nc.dram_tensor("w", (P, N), bf16, kind="ExternalInput").ap()
    o_d = nc.dram_tensor("out", (BC, N), fp32, kind="ExternalOutput").ap()

    add = mybir.AluOpType.add
    bypass = mybir.AluOpType.bypass

    with tile.TileContext(nc) as tc:
        with (
            tc.tile_pool(name="const", bufs=1) as cpool,
            tc.tile_pool(name="xin", bufs=GROUPS) as xpool,
            tc.tile_pool(name="work", bufs=4) as pool,
        ):
            # shared per-column weights, already broadcast across partitions;
            # issue on ACT's HWDGE queue so it is not behind the x DMAs
            wb = cpool.tile([P, N], bf16, tag="wb")
            nc.scalar.dma_start(out=wb, in_=w_d)

            # hoist all input loads: they have no dependencies, and the sync
            # DMA stream is in-order — emitting them first keeps later
            # groups' loads from queueing behind output dispatches.
            # Loads are split into column halves so the first compute can
            # start after half a transfer.
            half = 1024
            xts = []
            for g in range(GROUPS):
                xt = xpool.tile([P, M], fp32, tag="x")
                nc.sync.dma_start(
                    out=xt[:, :half], in_=x_d[g * P : (g + 1) * P, :half]
                )
                nc.sync.dma_start(
                    out=xt[:, half:], in_=x_d[g * P : (g + 1) * P, half:]
                )
                xts.append(xt)

            # ---- per-group pipeline: exp(-x) -> *w -> cumsum -> DMA out,
            # all stages in chained column halves for finer overlap
            for g in range(GROUPS):
                rows = slice(g * P, (g + 1) * P)
                xt = xts[g]
                et = pool.tile([P, N], bf16, tag="e")
                vt = pool.tile([P, N], bf16, tag="v")
                ut = pool.tile([P, N], fp32, tag="u")
                for h in range(2):
                    c0, c1 = h * half, (h + 1) * half if h == 0 else N
                    nc.scalar.activation(
                        out=et[:, c0:c1],
                        in_=xt[:, c0:c1],
                        func=mybir.ActivationFunctionType.Exp,
                        scale=-1.0,
                    )
                    # bf16 multiply runs in the DVE 2x perf mode
                    nc.vector.tensor_mul(
                        out=vt[:, c0:c1], in0=et[:, c0:c1], in1=wb[:, c0:c1]
                    )
                    # prefix sum along the grid dim; fp32 state and output
                    nc.vector.tensor_tensor_scan(
                        out=ut[:, c0:c1],
                        data0=vt[:, c0:c1],
                        data1=vt[:, c0:c1],
                        initial=0.0 if h == 0 else ut[:, c0 - 1 : c0],
                        op0=add,
                        op1=bypass,
                    )
                    nc.sync.dma_start(out=o_d[rows, c0:c1], in_=ut[:, c0:c1])

    nc.compile()
    return nc


def _get_nc():
    global _cached_nc
    if _cached_nc is None:
        _cached_nc = _build_kernel()
    return _cached_nc


def _make_w(f_rhs: np.ndarray) -> np.ndarray:
    """w = h2 * suffix_sum(f), w[0] halved; replicated to [P, N] bf16."""
    w = np.cumsum(f_rhs[::-1].astype(np.float64))[::-1] * H2
    w[0] *= 0.5
    wrow = w.astype(ml_dtypes.bfloat16)
    return np.ascontiguousarray(np.broadcast_to(wrow[None, :], (P, N)))


def kernel(x: np.ndarray, f_rhs: np.ndarray) -> np.ndarray:
    assert x.shape == (B, M) and f_rhs.shape == (N,)
    x = np.ascontiguousarray(x, dtype=np.float32)
    wb = _make_w(np.asarray(f_rhs, dtype=np.float32))
    nc = _get_nc()
    in_maps = [
        {"x": x[c * BC : (c + 1) * BC], "w": wb} for c in range(NCORES)
    ]
    res = bass_utils.run_bass_kernel_spmd(
        nc,
        in_maps,
        core_ids=list(range(NCORES)),
        trace=bool(int(os.environ.get("KERNEL_TRACE", "0"))),
    )
    global LAST_RESULTS
    LAST_RESULTS = res
    out = np.concatenate(
        [res.results[c]["out"] for c in range(NCORES)], axis=0
    ).astype(np.float32)
    return out


# revision 18
# speedup vs baseline: 1.0773x; 1.0773x over previous
"""Trainium2 Bass kernel for nn_DiffusionModel1d (batched 1-D diffusion solve).

Math: the reference solves A(K) u = f per batch row with K = exp(x) via the
Thomas algorithm, where A = G^T diag(K_hat) G, G the n x n lower-bidiagonal
difference matrix (1 on diag, -1 on subdiag) and
K_hat = (2*K_0, K_1, ..., K_{n-1}).  Hence

    u = h2 * G^{-1} diag(K_hat)^{-1} G^{-T} f
      = h2 * cumsum_j( w_j * exp(-x_j) ),   w = suffix_sum(f), w_0 halved.

So the whole solve is: one exp, one elementwise multiply by a shared
per-column vector, and one hardware prefix-sum scan along the grid dim.
Pure data parallel over batch: 8192 rows -> 1024 rows per core x 8 cores.
The tiny shared w vector (2047 elements, derived from the replicated f_rhs
by one suffix-sum pass) is prepared host-side and shipped replicated across
the 128 SBUF partitions, so the device pipeline has no serial prologue.

Engine budget per core (measured): DVE 2x bf16 mult 1.2us + 2 chained fp32
scans 2x2.3us per 128-row group (x8 groups), ACT exp ~1.5us, DMA 16.8 MB.
"""

import os
import sys

import numpy as np

sys.path.insert(0, "/opt/trn_rl_repo")

import ml_dtypes

import concourse.bacc as bacc
import concourse.mybir as mybir
import concourse.tile as tile
from concourse import bass_utils

B, M = 8192, 2048
N = M - 1
NCORES = 8
BC = B // NCORES          # 1024 batch rows per core
P = 128                   # SBUF partitions
GROUPS = BC // P          # 8 partition-groups per core
H2 = (1.0 / N) ** 2

_cached_nc = None
LAST_RESULTS = None


def _build_kernel():
    fp32 = mybir.dt.float32
    bf16 = mybir.dt.bfloat16
    nc = bacc.Bacc(
        "TRN2",
        target_bir_lowering=False,
        debug=False,
        enable_asserts=False,
        num_devices=NCORES,
    )
    x_d = nc.dram_tensor("x", (BC, M), fp32, kind="ExternalInput").ap()
    w_d = nc.dram_tensor("w", (P, N), bf16, kind="ExternalInput").ap()
    o_d = nc.dram_tensor("out", (BC, N), fp32, kind="ExternalOutput").ap()

    add = mybir.AluOpType.add
    bypass = mybir.AluOpType.bypass

    with tile.TileContext(nc) as tc:
        with (
            tc.tile_pool(name="const", bufs=1) as cpool,
            tc.tile_pool(name="xin", bufs=GROUPS) as xpool,
            tc.tile_pool(name="work", bufs=4) as pool,
        ):
            # shared per-column weights, already broadcast across partitions;
            # issue on ACT's HWDGE queue so it is not behind the x DMAs
            wb = cpool.tile([P, N], bf16, tag="wb")
            nc.scalar.dma_start(out=wb, in_=w_d)

            # hoist all input loads: they have no dependencies, and the sync
            # DMA stream is in-order — emitting them first keeps later
            # groups' loads from queueing behind output dispatches.
            # Group 0's load is split so its exp can start half a transfer
            # earlier (shorter ramp).
            half = 1024
            xts = []
            for g in range(GROUPS):
                xt = xpool.tile([P, M], fp32, tag="x")
                if g == 0:
                    nc.sync.dma_start(out=xt[:, :half], in_=x_d[:P, :half])
                    nc.sync.dma_start(out=xt[:, half:], in_=x_d[:P, half:])
                else:
                    nc.sync.dma_start(out=xt, in_=x_d[g * P : (g + 1) * P, :])
                xts.append(xt)

            # ---- per-group pipeline: exp(-x) -> *w -> cumsum -> DMA out.
            # Group 0's exp/mult run in column halves (shorter ramp); the
            # first and last groups' scan/store run in chained halves
            # (shorter ramp and tail).
            for g in range(GROUPS):
                rows = slice(g * P, (g + 1) * P)
                xt = xts[g]
                et = pool.tile([P, N], bf16, tag="e")
                vt = pool.tile([P, N], bf16, tag="v")
                ut = pool.tile([P, N], fp32, tag="u")
                em_splits = [(0, half), (half, N)] if g == 0 else [(0, N)]
                for c0, c1 in em_splits:
                    nc.scalar.activation(
                        out=et[:, c0:c1],
                        in_=xt[:, c0:c1],
                        func=mybir.ActivationFunctionType.Exp,
                        scale=-1.0,
                    )
                    nc.vector.tensor_mul(
                        out=vt[:, c0:c1], in0=et[:, c0:c1], in1=wb[:, c0:c1]
                    )
                su_splits = (
                    [(0, half), (half, N)] if g in (0, GROUPS - 1) else [(0, N)]
                )
                for si, (c0, c1) in enumerate(su_splits):
                    nc.vector.tensor_tensor_scan(
                        out=ut[:, c0:c1],
                        data0=vt[:, c0:c1],
                        data1=vt[:, c0:c1],
                        initial=0.0 if si == 0 else ut[:, c0 - 1 : c0],
                        op0=add,
                        op1=bypass,
                    )
                    nc.sync.dma_start(out=o_d[rows, c0:c1], in_=ut[:, c0:c1])

    nc.compile()
    return nc


def _get_nc():
    global _cached_nc
    if _cached_nc is None:
        _cached_nc = _build_kernel()
    return _cached_nc


def _make_w(f_rhs: np.ndarray) -> np.ndarray:
    """w = h2 * suffix_sum(f), w[0] halved; replicated to [P, N] bf16."""
    w = np.cumsum(f_rhs[::-1].astype(np.float64))[::-1] * H2
    w[0] *= 0.5
    wrow = w.astype(ml_dtypes.bfloat16)
    return np.ascontiguousarray(np.broadcast_to(wrow[None, :], (P, N)))


def kernel(x: np.ndarray, f_rhs: np.ndarray) -> np.ndarray:
    assert x.shape == (B, M) and f_rhs.shape == (N,)
    x = np.ascontiguousarray(x, dtype=np.float32)
    wb = _make_w(np.asarray(f_rhs, dtype=np.float32))
    nc = _get_nc()
    in_maps = [
        {"x": x[c * BC : (c + 1) * BC], "w": wb} for c in range(NCORES)
    ]
    res = bass_utils.run_bass_kernel_spmd(
        nc,
        in_maps,
        core_ids=list(range(NCORES)),
        trace=bool(int(os.environ.get("KERNEL_TRACE", "0"))),
    )
    global LAST_RESULTS
    LAST_RESULTS = res
    out = np.concatenate(
        [res.results[c]["out"] for c in range(NCORES)], axis=0
    ).astype(np.float32)
    return out
